# revision 1
# baseline (speedup 1.0000x reference)
"""Trainium2 Bass kernel for nn_CANet: 30-iteration neural cellular automaton.

cell [32,16,128,128] f32; per iteration:
    pre  = maxpool3x3(cell[:, :1]) > 0.1
    perc = [cell, dwconv(cell, sobel_x), dwconv(cell, sobel_y)]   # 48 ch
    h    = relu(w1 @ perc); cur = w2 @ h + cell
    post = maxpool3x3(cur[:, :1]) > 0.1
    cell = cur * (pre & post)
Output cell[:, 1:11] after 30 iterations.  Data parallel: 4 images/core x 8.

A layout: CELL [128p, 66*130]: p = 32q + 16g + ch, q = 2*pair + hh.
Image j = 2*pair + g; free = 66 rows x 130 cols (zero gutters + seam halos).
Ping-pong state (CELL_A/CELL_B) so evictions never conflict with tap reads.

Per iteration (src S -> dst D):
  taps: sobel+identity folded into 9 shifted-tap matmuls (K=32, M=96) per
        quarter, d-major weight-stationary order, four quarters concurrent
        on PE row-strips (tile_position=(32q,0)).
  relu -> H tiles; W2 via col-tiled (M=32 at col strip q) matmuls into a
  shared [128,512] cur bank; evict = scalar_tensor_tensor(cur + S) -> D
  (the +cell residual rides the eviction, one DVE op per 2048 px).
  Mask: cur ch0 spread to B layout [128p = 32j+hb, 6*130], maxpool+threshold
  on DVE, mask DMA'd back to a bf16 A-plane, broadcast to 16 channels by
  per-strip ones-matmuls (diag tiles) into a shared bank, applied by one
  tensor_tensor per 2048 px.
"""
import os
import sys

sys.path.insert(0, "/opt/trn_rl_repo")

import numpy as np

H = W = 128
C_IN = 10
CH = 16
NIMG = 4
NCORES = int(os.environ.get("CANET_CORES", "8"))
NITER = int(os.environ.get("CANET_NITER", "30"))
DT = os.environ.get("CANET_DT", "f32r")
ROWS = 66
COLS = 130
FREE = ROWS * COLS
NCH = 16                  # chunks per quarter
CR = 4                    # rows per chunk
N = CR * W                # 512
GC = 2                    # chunks per taps group
LIVE_THR = 0.1

_BUILT = None


def _sobel():
    s = np.array([[-1.0, 0.0, 1.0], [-2.0, 0.0, 2.0], [-1.0, 0.0, 1.0]],
                 dtype=np.float32) / 8.0
    return s, np.ascontiguousarray(s.T)


def _host_weights(w1, w2):
    import ml_dtypes
    sx, sy = _sobel()
    wts = np.zeros((128, 1120), np.float32)
    for dy in range(3):
        for dx in range(3):
            d = 3 * dy + dx
            F = w1[:, 16:32] * sx[dy, dx] + w1[:, 32:48] * sy[dy, dx]
            if dy == 1 and dx == 1:
                F = F + w1[:, 0:16]
            for q in range(4):
                for g in range(2):
                    wts[32 * q + 16 * g:32 * q + 16 * g + 16,
                        d * 96 + 48 * g:d * 96 + 48 * g + 48] = F.T
    # W2 col-block slices: separate fp32 blob [96K, 4x32M]
    wts2 = np.zeros((128, 128), np.float32)
    for g in range(2):
        for qp in range(4):
            wts2[48 * g:48 * g + 48,
                 32 * qp + 16 * g:32 * qp + 16 * g + 16] = w2.T
    ones = np.zeros((128, 128), np.float32)
    for q in range(4):
        for g in range(2):
            p = 32 * q + 16 * g
            ones[p, p:p + 16] = 1.0
    return wts, wts2, ones.astype(ml_dtypes.bfloat16)


def _host_cell0(inp4):
    cell = np.zeros((NIMG, CH, H, W), np.float32)
    cell[:, 1:C_IN + 1] = inp4
    cell[:, 0] = 1.0 - inp4[:, 0]
    dev = np.zeros((128, FREE), np.float32)
    for pair in range(2):
        for hh in range(2):
            q = 2 * pair + hh
            for g in range(2):
                j = 2 * pair + g
                for r in range(ROWS):
                    y = 64 * hh + r - 1
                    if 0 <= y < H:
                        dev[32 * q + 16 * g:32 * q + 16 * g + CH,
                            r * COLS + 1:r * COLS + 1 + W] = cell[j, :, y, :]
    return dev, cell


def _host_preb0(cell):
    import ml_dtypes
    x = cell[:, 0]
    p = np.full((NIMG, H + 2, W + 2), -np.inf, np.float32)
    p[:, 1:-1, 1:-1] = x
    mp = None
    for dy in range(3):
        for dx in range(3):
            win = p[:, dy:dy + H, dx:dx + W]
            mp = win.copy() if mp is None else np.maximum(mp, win)
    pre = (mp > LIVE_THR).astype(np.float32)
    out = np.zeros((128, 512), np.float32)
    for j in range(NIMG):
        for hb in range(32):
            out[32 * j + hb] = pre[j, 4 * hb:4 * hb + 4, :].reshape(-1)
    return out.astype(ml_dtypes.bfloat16)


def _split_waits(nc):
    import concourse.mybir as mybir
    nsplit = 0
    for fn in nc.m.functions:
        for bb in fn.blocks:
            new_list = []
            for ins in bb.instructions:
                si = ins.sync_info
                if si is not None and si.on_wait and len(si.on_wait) > 1:
                    waits = list(si.on_wait)
                    for w2 in waits[:-1]:
                        nop = mybir.InstNoOp(name=f"{ins.name}-ws{nsplit}")
                        nop.engine = ins.engine
                        nop.sync_info = mybir.SyncInfo(on_wait=[w2], on_update=[])
                        new_list.append(nop)
                        nsplit += 1
                    ins.sync_info = mybir.SyncInfo(on_wait=[waits[-1]],
                                                  on_update=list(si.on_update))
                new_list.append(ins)
            bb.instructions = new_list
    return nsplit


def _build():
    import concourse.bass as bass
    import concourse.mybir as mybir
    import concourse.tile as tile

    f32 = mybir.dt.float32
    bf16 = mybir.dt.bfloat16
    f32r = mybir.dt.float32r if DT == "f32r" else mybir.dt.float32

    nc = bass.Bass()
    cell0_in = nc.dram_tensor("cell0", [128, FREE], f32, kind="ExternalInput")
    wts_in = nc.dram_tensor("wts", [128, 1120], f32, kind="ExternalInput")
    wts2_in = nc.dram_tensor("wts2", [128, 128], f32, kind="ExternalInput")
    ones_in = nc.dram_tensor("ones", [128, 128], bf16, kind="ExternalInput")
    preb0_in = nc.dram_tensor("preb0", [128, 512], bf16, kind="ExternalInput")
    out_ext = nc.dram_tensor("out", [NIMG, C_IN, H, W], f32,
                             kind="ExternalOutput")

    AT = mybir.AluOpType

    with tile.TileContext(nc) as tc:
        with (
            tc.tile_pool(name="state", bufs=1) as sp,
            tc.tile_pool(name="hbuf", bufs=10) as hbp,
            tc.tile_pool(name="psum", bufs=8, space="PSUM") as pp,
        ):
            CELLA = sp.tile([128, FREE], f32r)
            CELLB = sp.tile([128, FREE], f32r)
            WTS = sp.tile([128, 1120], f32r)
            WTS2 = sp.tile([128, 128], f32)
            ONES = sp.tile([128, 128], bf16)
            MPLANE = sp.tile([128, FREE], bf16)
            B = sp.tile([128, 6 * COLS], f32)
            B2 = sp.tile([128, 6 * COLS], f32)
            PREB = sp.tile([128, 512], bf16)
            VM = sp.tile([128, 4 * COLS], f32)
            HM = sp.tile([128, 512], f32)
            POSTM = sp.tile([128, 512], bf16)
            MB = sp.tile([128, 512], bf16)

            with tc.tile_pool(name="init", bufs=1) as ip:
                C0 = ip.tile([128, FREE], f32)
                W0 = ip.tile([128, 1120], f32)
                nc.sync.dma_start(C0[:], cell0_in[:])
                nc.sync.dma_start(W0[:], wts_in[:])
                nc.vector.tensor_copy(CELLA[:], C0[:])
                nc.vector.tensor_copy(WTS[:], W0[:])
            nc.sync.dma_start(WTS2[:], wts2_in[:])
            nc.sync.dma_start(ONES[:], ones_in[:])
            nc.sync.dma_start(PREB[:], preb0_in[:])
            nc.gpsimd.memset(MPLANE[:], 0.0)
            nc.gpsimd.memset(B2[:], 0.0)
            nc.gpsimd.memset(CELLB[:].bitcast(f32), 0.0)

            mg = MPLANE[:].rearrange("p (r c) -> p r c", c=COLS)
            bg = B[:].rearrange("p (r c) -> p r c", c=COLS)
            b2g = B2[:].rearrange("p (r c) -> p r c", c=COLS)
            vg = VM[:].rearrange("p (r c) -> p r c", c=COLS)
            hmg = HM[:].rearrange("p (r c) -> p r c", c=W)

            def maxpool3(SRC, OUT):
                nc.vector.tensor_tensor(VM[:, :], SRC[:, 0:4 * COLS],
                                        SRC[:, COLS:5 * COLS], op=AT.max)
                nc.vector.tensor_tensor(VM[:, :], VM[:, :],
                                        SRC[:, 2 * COLS:6 * COLS], op=AT.max)
                nc.vector.tensor_tensor(hmg, vg[:, :, 0:W], vg[:, :, 1:1 + W],
                                        op=AT.max)
                nc.vector.tensor_tensor(hmg, hmg, vg[:, :, 2:2 + W], op=AT.max)
                nc.vector.tensor_single_scalar(OUT[:, :], HM[:, :], LIVE_THR,
                                               op=AT.is_gt)

            def iteration(S, D):
                """One CA step: read state S, write state D (tiles)."""
                sg = S[:].rearrange("p (r c) -> p r c", c=COLS)
                sgf = S[:].bitcast(f32).rearrange("p (r c) -> p r c", c=COLS)
                dg = D[:].rearrange("p (r c) -> p r c", c=COLS)
                dgf = D[:].bitcast(f32).rearrange("p (r c) -> p r c", c=COLS)

                # ---------- pass A: groups of GC chunks ----------
                for g0 in range(0, NCH, GC):
                    hps = {}
                    for ci in range(GC):
                        for q in range(4):
                            hp = pp.tile([128, N], f32, tag="ps")
                            hps[(q, g0 + ci)] = hp
                    # taps, d-major, weight-stationary per (d,q)
                    for d in range(9):
                        dy, dx = d // 3, d % 3
                        for q in range(4):
                            for ci in range(GC):
                                i = g0 + ci
                                r0 = 1 + CR * i
                                nc.tensor.matmul(
                                    hps[(q, i)][0:96, :],
                                    WTS[32 * q:32 * q + 32, d * 96:d * 96 + 96],
                                    sg[32 * q:32 * q + 32,
                                       r0 + dy - 1:r0 + dy - 1 + CR, dx:dx + W],
                                    start=(d == 0), stop=(d == 8),
                                    tile_position=(32 * q, 0))
                    # relu -> H tiles (frees tap banks)
                    Hs = {}
                    for ci in range(GC):
                        i = g0 + ci
                        for q in range(4):
                            Ht = hbp.tile([128, N], f32, tag="H")
                            Hs[(q, i)] = Ht
                            nc.any.tensor_relu(Ht[0:96, :], hps[(q, i)][0:96, :])
                    # W2: col-tiled (M=32 at strip q), q-outer c-inner
                    cps = {}
                    for ci in range(GC):
                        cp = pp.tile([128, N], f32, tag="ps")
                        cps[g0 + ci] = cp
                    for q in range(4):
                        for ci in range(GC):
                            i = g0 + ci
                            nc.tensor.matmul(
                                cps[i][32 * q:32 * q + 32, :],
                                WTS2[0:96, 32 * q:32 * q + 32],
                                Hs[(q, i)][0:96, :],
                                start=True, stop=True,
                                tile_position=(0, 32 * q))
                    # evict + residual: D = cur + S  (one op per chunk index)
                    for ci in range(GC):
                        i = g0 + ci
                        r0 = 1 + CR * i
                        nc.vector.scalar_tensor_tensor(
                            dg[:, r0:r0 + CR, 1:1 + W],
                            cps[i][:, :], 0.0,
                            sgf[:, r0:r0 + CR, 1:1 + W],
                            op0=AT.add, op1=AT.add)

                # ---------- seams on D (unmasked cur, for spread halos) -----
                def seam_dmas():
                    for pair in range(2):
                        q0, q1 = 2 * pair, 2 * pair + 1
                        nc.sync.dma_start(dg[32 * q1:32 * q1 + 32, 0, :],
                                          dg[32 * q0:32 * q0 + 32, 64, :])
                        nc.sync.dma_start(dg[32 * q0:32 * q0 + 32, 65, :],
                                          dg[32 * q1:32 * q1 + 32, 1, :])
                seam_dmas()
                # ---------- spread cur ch0 -> B ----------
                engs = [nc.sync, nc.sync]
                k = 0
                for pair in range(2):
                    for hh in range(2):
                        q = 2 * pair + hh
                        for g in range(2):
                            j = 2 * pair + g
                            p = 32 * q + 16 * g
                            srcb = D[p:p + 1].bitcast(f32)
                            src = bass.AP(
                                srcb.tensor, srcb.offset,
                                [list(srcb.ap[0]), [4 * COLS, 16], [COLS, 6],
                                 [1, COLS]])
                            engs[k % 2].dma_start(
                                B[32 * j + 16 * hh:32 * j + 16 * hh + 16, :]
                                .rearrange("p (r c) -> p r c", c=COLS), src)
                            k += 1
                # ---------- B-phase (critical path) ----------
                maxpool3(B, POSTM)
                nc.vector.tensor_tensor(MB[:, :], POSTM[:, :], PREB[:, :],
                                        op=AT.mult)
                # m back to A bf16 plane (gpsimd queues, overlap DVE tail)
                k = 0
                for pair in range(2):
                    for hh in range(2):
                        q = 2 * pair + hh
                        for g in range(2):
                            j = 2 * pair + g
                            p = 32 * q + 16 * g
                            nc.sync.dma_start(
                                mg[p:p + 1, 1:65, 1:1 + W],
                                MB[32 * j + 16 * hh:32 * j + 16 * hh + 16, :])
                            k += 1
                # broadcast mask (single K=128 bf16 matmul) and apply
                for i in range(NCH):
                    r0 = 1 + CR * i
                    mp = pp.tile([128, N], f32, tag="ps")
                    nc.tensor.matmul(
                        mp[:, :], ONES[:, :],
                        mg[:, r0:r0 + CR, 1:1 + W],
                        start=True, stop=True)
                    nc.any.tensor_tensor(
                        dg[:, r0:r0 + CR, 1:1 + W],
                        dgf[:, r0:r0 + CR, 1:1 + W],
                        mp[:, :], op=AT.mult)
                # ---------- off critical path: next-pre maxpool ----------
                nc.vector.tensor_tensor(
                    b2g[:, 1:5, 1:1 + W], bg[:, 1:5, 1:1 + W],
                    MB[:].rearrange("p (r c) -> p r c", c=W), op=AT.mult)
                for j in range(NIMG):
                    nc.sync.dma_start(b2g[32 * j:32 * j + 31, 5, :],
                                        b2g[32 * j + 1:32 * j + 32, 1, :])
                    nc.sync.dma_start(b2g[32 * j + 1:32 * j + 32, 0, :],
                                        b2g[32 * j:32 * j + 31, 4, :])
                maxpool3(B2, PREB)
                # ---------- seams with masked state ----------
                seam_dmas()

            if NITER == 1:
                iteration(CELLA, CELLB)
                FINAL = CELLB
            else:
                assert NITER % 2 == 0
                if os.environ.get("CANET_LOOP", "fori") == "fori" and NITER > 2:
                    with tc.For_i(0, NITER // 2, 1, hint_engines=(
                            mybir.EngineType.PE, mybir.EngineType.DVE,
                            mybir.EngineType.Activation, mybir.EngineType.SP,
                            mybir.EngineType.Pool)):
                        iteration(CELLA, CELLB)
                        iteration(CELLB, CELLA)
                else:
                    for it in range(NITER // 2):
                        iteration(CELLA, CELLB)
                        iteration(CELLB, CELLA)
                FINAL = CELLA

            fgf = FINAL[:].bitcast(f32).rearrange("p (r c) -> p r c", c=COLS)
            for pair in range(2):
                for hh in range(2):
                    q = 2 * pair + hh
                    for g in range(2):
                        j = 2 * pair + g
                        nc.sync.dma_start(
                            out_ext[j, :, 64 * hh:64 * hh + 64, :],
                            fgf[32 * q + 16 * g + 1:32 * q + 16 * g + 11,
                                1:65, 1:1 + W])

    nc.finalize()
    if os.environ.get("CANET_SPLIT", "1") == "1":
        _split_waits(nc)
    return nc


def _get_built():
    global _BUILT
    if _BUILT is None:
        _BUILT = _build()
    return _BUILT


def _host_inputs(inp, w1, w2):
    wts, wts2, ones = _host_weights(w1, w2)
    in_maps = []
    for core in range(NCORES):
        inp4 = inp[NIMG * core:NIMG * core + NIMG]
        cell0_dev, cell = _host_cell0(inp4)
        preb0 = _host_preb0(cell)
        in_maps.append({"cell0": cell0_dev, "wts": wts, "wts2": wts2,
                        "ones": ones, "preb0": preb0})
    return in_maps


def kernel(inp, w1, w2):
    inp = np.asarray(inp, np.float32)
    w1 = np.asarray(w1, np.float32)
    w2 = np.asarray(w2, np.float32)
    from concourse.bass_utils import run_bass_kernel_spmd

    nc = _get_built()
    in_maps = _host_inputs(inp, w1, w2)
    res = run_bass_kernel_spmd(nc, in_maps, list(range(NCORES))).results
    out = np.concatenate([res[c]["out"] for c in range(NCORES)], axis=0)
    return np.ascontiguousarray(out.astype(np.float32))

